# revision 1
# baseline (speedup 1.0000x reference)
"""Trainium2 Bass kernel for Performer-style (FAVOR+) causal linear attention.

Reference computation (per batch b=1, heads h=16, seq s=2048, d=64, r=64):
  qh = split_heads((q @ wq + bq) * d^-0.25)     kh likewise, vh = split_heads(v @ wv + bv)
  q' = (1/sqrt(d)) * exp(qh @ wg - 0.5*||qh||^2)   k' likewise
  attn[s] = (q'_s . sum_{j<=s} k'_j v_j^T) / (eps + q'_s . sum_{j<=s} k'_j)
  out = merge_heads(attn) @ wc + bc

Sharding: 2 heads per core (16 heads over 8 cores). Each core receives the
full (transposed, fp16) q/k/v plus its 128-column slice of the projection
weights, computes its heads' attention via a chunked causal scan (chunk=128),
projects through its 128-row slice of wc, and returns a (2048, 1024) fp16
partial. The host sums the 8 partials and adds the output bias.
"""

import sys

if "/opt/trn_rl_repo" not in sys.path:
    sys.path.insert(0, "/opt/trn_rl_repo")

import math
from contextlib import ExitStack

import numpy as np

D_MODEL = 1024
N_HEADS = 16
D = 64  # head depth
R = 64  # kernel features
S = 2048
N_CORES = 8
HPC = N_HEADS // N_CORES  # heads per core = 2
CW = HPC * D  # per-core channel width = 128
P = 128
ST = 512  # projection s-tile width
NST = S // ST  # 4
C = 128  # scan chunk
NCH = S // C  # 16
KT = D_MODEL // P  # 8 contraction tiles
NORM_D = float(D ** (-0.25))
LN_RSQRT_D = float(-0.5 * math.log(D))  # exp(x + this) = exp(x)/sqrt(d)

_CACHE = {}


def _build_bass(nst=NST, nch=NCH, stage=9):
    import concourse.bass as bass
    import concourse.mybir as mybir
    import concourse.tile as tile
    from concourse.bacc import Bacc

    f16 = mybir.dt.float16
    f32 = mybir.dt.float32
    AF = mybir.ActivationFunctionType
    Alu = mybir.AluOpType

    nc = Bacc(trn_type="TRN2")

    qT = nc.dram_tensor("qT", [D_MODEL, S], f16, kind="ExternalInput")
    kT = nc.dram_tensor("kT", [D_MODEL, S], f16, kind="ExternalInput")
    vT = nc.dram_tensor("vT", [D_MODEL, S], f16, kind="ExternalInput")
    wq = nc.dram_tensor("wq", [D_MODEL, CW], f16, kind="ExternalInput")
    wk = nc.dram_tensor("wk", [D_MODEL, CW], f16, kind="ExternalInput")
    wv = nc.dram_tensor("wv", [D_MODEL, CW], f16, kind="ExternalInput")
    # aux: [ident(128) | mask(128) | wg2(64) | ng2(64)] packed along free dim
    aux = nc.dram_tensor("aux", [P, 2 * P + 2 * R], f16, kind="ExternalInput")
    bqkv = nc.dram_tensor("bqkv", [CW, 3], f32, kind="ExternalInput")
    wc = nc.dram_tensor("wc", [CW, D_MODEL], f16, kind="ExternalInput")
    out = nc.dram_tensor("out", [S, D_MODEL], f16, kind="ExternalOutput")

    with tile.TileContext(nc) as tc, ExitStack() as ctx:
        # ---- constant / weight tiles ----
        const = ctx.enter_context(tc.tile_pool(name="const", bufs=1))
        w_sb = {}
        for name, drt in (("wq", wq), ("wk", wk), ("wv", wv)):
            t = const.tile([P, KT * CW], f16, tag=name, name=f"wt_{name}")
            # dest[p, k*CW + c] <- w[k*P + p, c]
            dst = t[:].rearrange("p (k c) -> p k c", k=KT)
            sr = drt[:, :].rearrange("(k p) c -> p k c", p=P)
            nc.sync.dma_start(dst, sr)
            for k in range(KT):
                w_sb[(name, k)] = t[:, k * CW : (k + 1) * CW]
        aux_sb = const.tile([P, 2 * P + 2 * R], f16, tag="aux")
        nc.sync.dma_start(aux_sb[:], aux[:, :])
        id_sb = aux_sb[:, 0:P]
        mask_sb = aux_sb[:, P : 2 * P]
        wg_sb = aux_sb[:, 2 * P : 2 * P + R]
        ng_sb = aux_sb[:, 2 * P + R : 2 * P + 2 * R]
        wc_sb = const.tile([CW, D_MODEL], f16, tag="wc")
        nc.sync.dma_start(wc_sb[:], wc[:, :])
        b_all = const.tile([CW, 3], f32, tag="ball")
        nc.sync.dma_start(b_all[:], bqkv[:, :])
        b_sb = {"bq": b_all[:, 0:1], "bk": b_all[:, 1:2], "bv": b_all[:, 2:3]}
        ebias = const.tile([P, 1], f32, tag="ebias")
        nc.vector.memset(ebias[:], LN_RSQRT_D)

        # ---- pools ----
        xin = ctx.enter_context(tc.tile_pool(name="xin", bufs=24))
        tmp_pool = ctx.enter_context(tc.tile_pool(name="tmp", bufs=2))
        big_psum = ctx.enter_context(tc.tile_pool(name="bigp", bufs=2, space="PSUM"))
        prj_psum = big_psum
        phi_psum = big_psum
        qp_pool = ctx.enter_context(tc.tile_pool(name="qp", bufs=NST))
        kp_pool = ctx.enter_context(tc.tile_pool(name="kp", bufs=NST))
        vh_pool = ctx.enter_context(tc.tile_pool(name="vh", bufs=NST))

        # stream inputs: one DMA per (tensor, k-tile, s-half); first halves first
        x_sb = {}
        for name, srct in (("q", qT), ("k", kT), ("v", vT)):
            for k in range(KT):
                x_sb[(name, k)] = xin.tile([P, S], f16, tag="xin", name=f"x_{name}{k}")
        H = S // 2
        for half in range(2):
            for name, srct in (("q", qT), ("k", kT), ("v", vT)):
                for k in range(KT):
                    nc.sync.dma_start(
                        x_sb[(name, k)][:, half * H : (half + 1) * H],
                        srct[k * P : (k + 1) * P, half * H : (half + 1) * H],
                    )

        # per s-tile: projections for q, k, v + feature maps for q, k
        qp_t, kp_t, vh_t = [], [], []

        def emit_stile(st):
            sl = slice(st * ST, (st + 1) * ST)
            for name in ("q", "k", "v"):
                pp = prj_psum.tile([P, ST], f32, tag="big", name=f"prj_{st}_{name}")
                for k in range(KT):
                    nc.tensor.matmul(
                        pp[:], w_sb[("w" + name, k)][:], x_sb[(name, k)][:, sl],
                        start=(k == 0), stop=(k == KT - 1)
                    )
                if name == "v":
                    vh = vh_pool.tile([P, ST], f16, tag="vh")
                    # vh = psum + bv
                    nc.vector.tensor_scalar(vh[:], pp[:], b_sb["bv"][:], None, Alu.add)
                    vh_t.append(vh)
                else:
                    # tmp = psum * NORM_D + b  (b pre-scaled by NORM_D on host)
                    tmp = tmp_pool.tile([P, ST], f16, tag="tmpl")
                    nc.vector.tensor_scalar(
                        tmp[:], pp[:], NORM_D, b_sb["b" + name][:], Alu.mult, Alu.add
                    )
                    tmp2 = tmp_pool.tile([P, ST], f16, tag="tmps")
                    nc.vector.tensor_tensor(tmp2[:], tmp[:], tmp[:], Alu.mult)
                    fp = phi_psum.tile([P, ST], f32, tag="big", name=f"phi_{st}_{name}")
                    nc.tensor.matmul(fp[0:D, :], wg_sb[0:D, :], tmp[0:D, :], start=True, stop=False)
                    nc.tensor.matmul(fp[0:D, :], ng_sb[0:D, :], tmp2[0:D, :], start=False, stop=True)
                    nc.tensor.matmul(
                        fp[D:P, :], wg_sb[D:P, :], tmp[D:P, :],
                        start=True, stop=False, tile_position=(D, D),
                    )
                    nc.tensor.matmul(
                        fp[D:P, :], ng_sb[D:P, :], tmp2[D:P, :],
                        start=False, stop=True, tile_position=(D, D),
                    )
                    dst_pool = qp_pool if name == "q" else kp_pool
                    pt = dst_pool.tile([P, ST], f16, tag="qkp")
                    nc.scalar.activation(pt[:], fp[:], AF.Exp, bias=ebias[:])
                    (qp_t if name == "q" else kp_t).append(pt)

        # ---- attention scan (chunk = 128) ----
        tp_psum = ctx.enter_context(tc.tile_pool(name="tpp", bufs=2, space="PSUM"))
        at_psum = ctx.enter_context(tc.tile_pool(name="atp", bufs=1, space="PSUM"))
        o_psum = ctx.enter_context(tc.tile_pool(name="op", bufs=1, space="PSUM"))
        s_psum = ctx.enter_context(tc.tile_pool(name="sp", bufs=1, space="PSUM"))
        ot_psum = tp_psum
        f_psum = ctx.enter_context(tc.tile_pool(name="fpp", bufs=1, space="PSUM"))
        sc_pool = ctx.enter_context(tc.tile_pool(name="sc", bufs=6))
        ot_pool = ctx.enter_context(tc.tile_pool(name="ot", bufs=8))
        out_pool = ctx.enter_context(tc.tile_pool(name="outp", bufs=16))

        s_ps = s_psum.tile([P, D + 1], f32, tag="S")
        # persistent V_aug tiles (even/odd) with ones columns at 64 and 129
        vaug = []
        s_sb = []
        for par in range(2):
            va = const.tile([P, 2 * (D + 1)], f16, tag=f"vaug{par}")
            ones_ap = va[:].rearrange("p (b c) -> p b c", c=D + 1)[:, :, D]
            nc.vector.memset(ones_ap, 1.0)
            vaug.append(va)
            s_sb.append(const.tile([P, D + 1], f16, tag=f"ssb{par}", name=f"ssb{par}"))

        def emit_chunk(c):
            if stage < 2:
                return
            st, off = c // 4, (c % 4) * C
            csl = slice(off, off + C)
            va = vaug[c % 2]
            # K' and V transposed to s-major via PE transpose
            ktp = tp_psum.tile([P, P], f16, tag="tp")
            nc.tensor.transpose(ktp[:], kp_t[st][:, csl], id_sb[:])
            ks = sc_pool.tile([P, P], f16, tag="ks")
            nc.vector.tensor_copy(ks[:], ktp[:])
            vtp = tp_psum.tile([P, P], f16, tag="tp")
            nc.tensor.transpose(vtp[:], vh_t[st][:, csl], id_sb[:])
            va_dst = va[:].rearrange("p (b c) -> p b c", c=D + 1)[:, :, 0:D]
            nc.scalar.activation(va_dst, vtp[:].rearrange("p (b c) -> p b c", c=D), AF.Copy)

            if stage < 3:
                return
            # intra-chunk attention AT[j,i] per head (row-packed pair)
            atm = []
            for h in range(HPC):
                atp = at_psum.tile([P, P], f32, tag="at", name=f"at{h}_{c}")
                nc.tensor.matmul(
                    atp[:], kp_t[st][h * D : (h + 1) * D, csl],
                    qp_t[st][h * D : (h + 1) * D, csl],
                    tile_position=(h * D, 0), start=True, stop=True,
                )
                am = sc_pool.tile([P, P], f16, tag=f"atm{h}", name=f"atm{h}_{c}")
                nc.vector.tensor_tensor(am[:], atp[:], mask_sb[:], Alu.mult)
                atm.append(am)

            if stage < 4:
                return
            # O psum (i, [attn_h | qk_h] x2): intra + inter contributions
            ops = []
            for h in range(HPC):
                oph = o_psum.tile([P, D + 1], f32, tag="o", name=f"o{h}_{c}")
                nc.tensor.matmul(
                    oph[:], atm[h][:], va[:, h * (D + 1) : (h + 1) * (D + 1)],
                    start=True, stop=(c == 0),
                )
                if c > 0:
                    nc.tensor.matmul(
                        oph[:], qp_t[st][h * D : (h + 1) * D, csl],
                        s_sb[c % 2][h * D : (h + 1) * D, :],
                        start=False, stop=True,
                    )
                ops.append(oph)

            if stage < 5:
                return
            # state update S += K'_s^T-outer  (col-packed pair), then copy for next chunk
            for h in range(HPC):
                nc.tensor.matmul(
                    s_ps[h * D : (h + 1) * D, :], ks[:, h * D : (h + 1) * D],
                    va[:, h * (D + 1) : (h + 1) * (D + 1)],
                    tile_position=(0, h * D),
                    start=(c == 0), stop=(c == nch - 1),
                    skip_group_check=True,
                )
            if c < nch - 1:
                nc.scalar.activation(s_sb[(c + 1) % 2][:], s_ps[:], AF.Copy)

            if stage < 6:
                return
            # normalize: recip of qk columns (64, 129), scale, transpose back
            rc = sc_pool.tile([P, HPC], f32, tag="rc")
            for h in range(HPC):
                nc.vector.reciprocal(rc[:, h : h + 1], ops[h][:, D : D + 1])
            osb = sc_pool.tile([P, P], f16, tag="osb")
            for h in range(HPC):
                nc.vector.tensor_scalar(
                    osb[:, h * D : (h + 1) * D], ops[h][:, 0:D],
                    rc[:, h : h + 1], None, Alu.mult,
                )
            otp = at_psum.tile([P, P], f16, tag="at", name=f"otp_{c}")
            nc.tensor.transpose(otp[:], osb[:], id_sb[:])
            ott = ot_pool.tile([P, P], f16, tag="ott")
            nc.scalar.activation(ott[:], otp[:], AF.Copy)

            if stage < 7:
                return
            # final projection for this chunk + store
            ob = out_pool.tile([P, D_MODEL], f16, tag="ob")
            fps0 = f_psum.tile([P, ST], f32, tag="f", name=f"f0_{c}")
            nc.tensor.matmul(fps0[:], ott[:], wc_sb[:, 0:ST], start=True, stop=True)
            fps1 = f_psum.tile([P, ST], f32, tag="f", name=f"f1_{c}")
            nc.tensor.matmul(fps1[:], ott[:], wc_sb[:, ST:D_MODEL], start=True, stop=True)
            if c % 2 == 0:
                nc.scalar.activation(ob[:, 0:ST], fps0[:], AF.Copy)
                nc.scalar.activation(ob[:, ST:D_MODEL], fps1[:], AF.Copy)
            else:
                nc.vector.tensor_copy(ob[:, 0:ST], fps0[:])
                nc.vector.tensor_copy(ob[:, ST:D_MODEL], fps1[:])
            nc.sync.dma_start(out[c * C : (c + 1) * C, :], ob[:])


        for st in range(nst):
            emit_stile(st)
            for c in range(4 * st, min(4 * st + 4, nch)):
                emit_chunk(c)

    nc.finalize()
    return nc


def _prep_inputs(v, k, q, wq_w, wq_b, wk_w, wk_b, wv_w, wv_b, wc_w, wc_b, wg):
    f16 = np.float16
    qT = np.ascontiguousarray(q[0].T).astype(f16)
    kT = np.ascontiguousarray(k[0].T).astype(f16)
    vT = np.ascontiguousarray(v[0].T).astype(f16)
    wg2 = np.concatenate([wg, wg], axis=0).astype(f16)  # (128, 64)
    ng2 = np.full((P, R), -0.5, f16)
    ident = np.eye(P, dtype=f16)
    mask = np.triu(np.ones((P, P), np.float32)).astype(f16)  # mask[j,i]=1 iff j<=i
    aux = np.concatenate([ident, mask, wg2, ng2], axis=1)  # (128, 384)
    in_maps = []
    for c in range(N_CORES):
        cs = slice(c * CW, (c + 1) * CW)
        bqkv = np.stack([
            (wq_b[cs] * NORM_D).astype(np.float32),
            (wk_b[cs] * NORM_D).astype(np.float32),
            wv_b[cs].astype(np.float32),
        ], axis=1)
        in_maps.append({
            "qT": qT, "kT": kT, "vT": vT,
            "wq": wq_w[:, cs].astype(f16),
            "wk": wk_w[:, cs].astype(f16),
            "wv": wv_w[:, cs].astype(f16),
            "bqkv": bqkv,
            "aux": aux,
            "wc": wc_w[cs, :].astype(f16),
        })
    return in_maps


def kernel(**inputs):
    from concourse.bass_utils import run_bass_kernel_spmd

    if "nc" not in _CACHE:
        _CACHE["nc"] = _build_bass()
    nc = _CACHE["nc"]
    in_maps = _prep_inputs(**inputs)
    res = run_bass_kernel_spmd(nc, in_maps, core_ids=list(range(N_CORES)))
    _CACHE["last_results"] = res
    acc = np.zeros((S, D_MODEL), np.float32)
    for c in range(N_CORES):
        acc += res.results[c]["out"].astype(np.float32)
    acc += inputs["wc_b"].astype(np.float32)[None, :]
    return acc[None, :, :]


if __name__ == "__main__":
    import reference

    inp = {k: np.asarray(v) for k, v in reference.setup_inputs().items()}
    got = kernel(**inp)
    print("kernel out", got.shape, got.dtype)



# revision 7
# speedup vs baseline: 1.0873x; 1.0873x over previous
"""Trainium2 Bass kernel for Performer-style (FAVOR+) causal linear attention.

Reference computation (per batch b=1, heads h=16, seq s=2048, d=64, r=64):
  qh = split_heads((q @ wq + bq) * d^-0.25)     kh likewise, vh = split_heads(v @ wv + bv)
  q' = (1/sqrt(d)) * exp(qh @ wg - 0.5*||qh||^2)   k' likewise
  attn[s] = (q'_s . sum_{j<=s} k'_j v_j^T) / (eps + q'_s . sum_{j<=s} k'_j)
  out = merge_heads(attn) @ wc + bc

Key algebraic simplification: wg is orthogonal (64x64 from QR), so
||qh||^2 == ||qh @ wg||^2. Folding wg into the projection weights
(wqg = norm * wq @ blockdiag(wg)) means the kernel only ever computes
qhg = q @ wqg, and q' = exp(qhg) * exp(-0.5*sum_r qhg_r^2) / sqrt(d).

Sharding: 2 heads per core (16 heads over 8 cores). Each core receives the
full (transposed, fp16) q/k/v plus its 128-column slice of the (folded)
projection weights, computes its heads' attention via a de-serialized
chunked causal scan (chunk=128, per-chunk states + vector-engine prefix
sum), projects through its 128-row slice of wc, and returns a
(2048, 1024) fp16 partial. The host sums the 8 partials and adds wc_b.

Schedule: inputs stream st-major (3MB per 512-token s-tile); each s-tile's
projections + feature maps are emitted interleaved with the previous
s-tile's four scan chunks so the tensor queue stays fed while DMA streams.
"""

import sys

if "/opt/trn_rl_repo" not in sys.path:
    sys.path.insert(0, "/opt/trn_rl_repo")

import math
from contextlib import ExitStack

import numpy as np

D_MODEL = 1024
N_HEADS = 16
D = 64  # head depth
R = 64  # kernel features (= D, wg orthogonal)
S = 2048
N_CORES = 8
HPC = N_HEADS // N_CORES  # heads per core = 2
CW = HPC * D  # per-core channel width = 128
P = 128
ST = 512  # projection s-tile width
NST = S // ST  # 4
C = 128  # scan chunk
NCH = S // C  # 16
KT = D_MODEL // P  # 8 contraction tiles
W = D + 1  # augmented value width (v | 1)
NORM_D = float(D ** (-0.25))
LN_RSQRT_D = float(-0.5 * math.log(D))  # exp(x + this) = exp(x)/sqrt(d)

_CACHE = {}


def _build_bass():
    import concourse.bass as bass
    import concourse.mybir as mybir
    import concourse.tile as tile
    from concourse.bacc import Bacc

    f16 = mybir.dt.float16
    f32 = mybir.dt.float32
    AF = mybir.ActivationFunctionType
    Alu = mybir.AluOpType

    nc = Bacc(trn_type="TRN2")

    qT = nc.dram_tensor("qT", [D_MODEL, S], f16, kind="ExternalInput")
    kT = nc.dram_tensor("kT", [D_MODEL, S], f16, kind="ExternalInput")
    vT = nc.dram_tensor("vT", [D_MODEL, S], f16, kind="ExternalInput")
    wq = nc.dram_tensor("wq", [D_MODEL, CW], f16, kind="ExternalInput")
    wk = nc.dram_tensor("wk", [D_MODEL, CW], f16, kind="ExternalInput")
    wv = nc.dram_tensor("wv", [D_MODEL, CW], f16, kind="ExternalInput")
    # aux: [ident(128) | mask(128) | ng(64)] packed along free dim
    aux = nc.dram_tensor("aux", [P, 2 * P + R], f16, kind="ExternalInput")
    bqkv = nc.dram_tensor("bqkv", [CW, 3], f32, kind="ExternalInput")
    wc = nc.dram_tensor("wc", [CW, D_MODEL], f16, kind="ExternalInput")
    out = nc.dram_tensor("out", [S, D_MODEL], f16, kind="ExternalOutput")

    with tile.TileContext(nc) as tc, ExitStack() as ctx:
        # ---- constant / weight tiles (DMA on scalar queue; x stream on sync) ----
        const = ctx.enter_context(tc.tile_pool(name="const", bufs=1))
        w_sb = {}
        for name, drt in (("wq", wq), ("wk", wk), ("wv", wv)):
            t = const.tile([P, KT * CW], f16, tag=name, name=f"wt_{name}")
            dst = t[:].rearrange("p (k c) -> p k c", k=KT)
            sr = drt[:, :].rearrange("(k p) c -> p k c", p=P)
            nc.scalar.dma_start(dst, sr)
            for k in range(KT):
                w_sb[(name, k)] = t[:, k * CW : (k + 1) * CW]
        b_all = const.tile([CW, 3], f32, tag="ball")
        nc.scalar.dma_start(b_all[:], bqkv[:, :])
        b_sb = {"bq": b_all[:, 0:1], "bk": b_all[:, 1:2], "bv": b_all[:, 2:3]}
        aux_sb = const.tile([P, 2 * P + R], f16, tag="aux")
        nc.scalar.dma_start(aux_sb[:], aux[:, :])
        id_sb = aux_sb[:, 0:P]
        mask_sb = aux_sb[:, P : 2 * P]
        ng_sb = aux_sb[:, 2 * P : 2 * P + R]
        wc_sb = const.tile([CW, D_MODEL], f16, tag="wc")
        nc.scalar.dma_start(wc_sb[:], wc[:, :])
        ebias = const.tile([P, 1], f32, tag="ebias")
        nc.vector.memset(ebias[:], LN_RSQRT_D)

        # persistent per-chunk V tiles with ones column at 64/129
        va_t = []
        for c in range(NCH):
            va = const.tile([P, HPC * W], f16, tag=f"va{c}", name=f"va{c}")
            ones_ap = va[:].rearrange("p (b c) -> p b c", c=W)[:, :, D]
            nc.vector.memset(ones_ap, 1.0)
            va_t.append(va)

        # ---- x input tiles, DMA'd st-major: (q,k,v) x st, 1MB per DMA ----
        xin = ctx.enter_context(tc.tile_pool(name="xin", bufs=1))
        x_t = {}
        for name in ("q", "k", "v"):
            x_t[name] = xin.tile([P, KT * S], f16, tag=f"x_{name}", name=f"x_{name}")
        for st in range(NST):
            sl = slice(st * ST, (st + 1) * ST)
            for name, srct in (("q", qT), ("k", kT), ("v", vT)):
                dst = x_t[name][:].rearrange("p (k s) -> p k s", k=KT)[:, :, sl]
                sr = srct[:, sl].rearrange("(k p) s -> p k s", p=P)
                nc.sync.dma_start(dst, sr)

        def xs(name, k, st):
            return x_t[name][:, k * S + st * ST : k * S + (st + 1) * ST]

        # ---- pools ----
        tmp_pool = ctx.enter_context(tc.tile_pool(name="tmp", bufs=3))
        # PSUM: 8 banks x 2KB/partition: bigp(3) + tpp(2) + sp(1) + atp(1) + op(1)
        big_psum = ctx.enter_context(tc.tile_pool(name="bigp", bufs=3, space="PSUM"))
        tp_psum = ctx.enter_context(tc.tile_pool(name="tpp", bufs=2, space="PSUM"))
        s_psum = ctx.enter_context(tc.tile_pool(name="sp", bufs=1, space="PSUM"))
        at_psum = ctx.enter_context(tc.tile_pool(name="atp", bufs=1, space="PSUM"))
        o_psum = ctx.enter_context(tc.tile_pool(name="op", bufs=1, space="PSUM"))
        qp_pool = ctx.enter_context(tc.tile_pool(name="qp", bufs=NST))
        kp_pool = ctx.enter_context(tc.tile_pool(name="kp", bufs=NST))
        vh_pool = ctx.enter_context(tc.tile_pool(name="vh", bufs=NST))
        sc_pool = ctx.enter_context(tc.tile_pool(name="sc", bufs=4))
        prefF_pool = ctx.enter_context(tc.tile_pool(name="prF", bufs=2))
        pref16_pool = ctx.enter_context(tc.tile_pool(name="pr16", bufs=3))
        ot_pool = ctx.enter_context(tc.tile_pool(name="ot", bufs=3))
        out_pool = ctx.enter_context(tc.tile_pool(name="outp", bufs=3))

        qp_t, kp_t, vh_t = [], [], []
        s_ps = [None] * NCH
        prefF = [None] * (NCH + 1)
        pref16 = [None] * (NCH + 1)
        atm_t = [None] * NCH

        def emit_proj(name, st):
            pp = big_psum.tile([P, ST], f32, tag="big", name=f"prj_{name}{st}")
            for k in range(KT):
                nc.tensor.matmul(
                    pp[:], w_sb[("w" + name, k)][:], xs(name, k, st),
                    start=(k == 0), stop=(k == KT - 1)
                )
            return pp

        def emit_post_v(pp, st):
            vh = vh_pool.tile([P, ST], f16, tag="vh")
            nc.vector.tensor_scalar(vh[:], pp[:], b_sb["bv"][:], None, Alu.add)
            vh_t.append(vh)

        def emit_tmp(name, pp, st):
            """qhg psum -> tmp fp16 (+bias) and tmp2 = tmp^2 (vector)."""
            tmp = tmp_pool.tile([P, ST], f16, tag=f"tmpl_{name}")
            nc.vector.tensor_scalar(tmp[:], pp[:], b_sb["b" + name][:], None, Alu.add)
            tmp2 = tmp_pool.tile([P, ST], f16, tag=f"tmps_{name}")
            nc.vector.tensor_tensor(tmp2[:], tmp[:], tmp[:], Alu.mult)
            return tmp, tmp2

        def emit_feat(name, tmp, tmp2, st):
            """q' = exp(qhg) * exp(-0.5*sum_d qhg^2 + ln(1/sqrt d))."""
            fp = big_psum.tile([P, ST], f32, tag="big", name=f"phi_{name}{st}")
            nc.tensor.matmul(fp[0:D, :], ng_sb[0:D, :], tmp2[0:D, :],
                             start=True, stop=True)
            nc.tensor.matmul(fp[D:P, :], ng_sb[D:P, :], tmp2[D:P, :],
                             start=True, stop=True, tile_position=(D, D))
            e1 = tmp_pool.tile([P, ST], f16, tag=f"e1_{name}")
            nc.scalar.activation(e1[:], tmp[:], AF.Exp)
            e2 = tmp_pool.tile([P, ST], f16, tag=f"e2_{name}")
            nc.scalar.activation(e2[:], fp[:], AF.Exp, bias=ebias[:])
            dst_pool = qp_pool if name == "q" else kp_pool
            pt = dst_pool.tile([P, ST], f16, tag="qkp")
            nc.vector.tensor_tensor(pt[:], e1[:], e2[:], Alu.mult)
            (qp_t if name == "q" else kp_t).append(pt)

        def emit_indep(c):
            """Transposes, per-chunk state, prefix step (independent work)."""
            st, off = c // 4, (c % 4) * C
            csl = slice(off, off + C)
            va = va_t[c]
            # K' transposed to s-major
            ktp = tp_psum.tile([P, P], f16, tag="tp")
            nc.tensor.transpose(ktp[:], kp_t[st][:, csl], id_sb[:])
            ks = sc_pool.tile([P, P], f16, tag="ks")
            nc.vector.tensor_copy(ks[:], ktp[:])
            # V transposed to s-major into persistent va tile
            vtp = tp_psum.tile([P, P], f16, tag="tp")
            nc.tensor.transpose(vtp[:], vh_t[st][:, csl], id_sb[:])
            va_dst = va[:].rearrange("p (b c) -> p b c", c=W)[:, :, 0:D]
            nc.scalar.activation(va_dst, vtp[:].rearrange("p (b c) -> p b c", c=D), AF.Copy)
            # per-chunk state S_c[r, d|1] (2 heads packed on partitions)
            sp = s_psum.tile([P, W], f32, tag="S", name=f"S{c}")
            for h in range(HPC):
                nc.tensor.matmul(
                    sp[h * D : (h + 1) * D, :], ks[:, h * D : (h + 1) * D],
                    va[:, h * W : (h + 1) * W],
                    tile_position=(0, h * D), start=True, stop=True,
                    skip_group_check=True,
                )
            s_ps[c] = sp
            # prefix: pref[c+1] = pref[c] + S_c (f32 vector; f16 copy on gpsimd)
            pf = prefF_pool.tile([P, W], f32, tag="prF")
            if c == 0:
                nc.vector.tensor_copy(pf[:], sp[:])
            else:
                nc.vector.tensor_tensor(pf[:], prefF[c][:], sp[:], Alu.add)
            prefF[c + 1] = pf
            p16 = pref16_pool.tile([P, W], f16, tag="pr16")
            nc.gpsimd.tensor_copy(p16[:], pf[:])
            pref16[c + 1] = p16

        def emit_at(c):
            """Intra-chunk attention matrix + mask (both heads in one bank)."""
            st, off = c // 4, (c % 4) * C
            csl = slice(off, off + C)
            atp = at_psum.tile([P, 2 * P], f32, tag="at", name=f"at_{c}")
            atm = []
            for h in range(HPC):
                nc.tensor.matmul(
                    atp[:, h * P : (h + 1) * P],
                    kp_t[st][h * D : (h + 1) * D, csl],
                    qp_t[st][h * D : (h + 1) * D, csl],
                    tile_position=(h * D, 0), start=True, stop=True,
                    skip_group_check=True,
                )
                am = sc_pool.tile([P, P], f16, tag=f"atm{h}", name=f"atm{h}_{c}")
                nc.vector.tensor_tensor(
                    am[:], atp[:, h * P : (h + 1) * P], mask_sb[:], Alu.mult
                )
                atm.append(am)
            atm_t[c] = atm

        def emit_o(c):
            """O = intra + inter, normalize, transpose, final projection, store."""
            st, off = c // 4, (c % 4) * C
            csl = slice(off, off + C)
            va = va_t[c]
            op_t = o_psum.tile([P, HPC * W], f32, tag="o", name=f"o_{c}")
            ops = []
            for h in range(HPC):
                oph = op_t[:, h * W : (h + 1) * W]
                nc.tensor.matmul(
                    oph, atm_t[c][h][:], va[:, h * W : (h + 1) * W],
                    start=True, stop=(c == 0), skip_group_check=True,
                )
                if c > 0:
                    nc.tensor.matmul(
                        oph, qp_t[st][h * D : (h + 1) * D, csl],
                        pref16[c][h * D : (h + 1) * D, :],
                        start=False, stop=True, skip_group_check=True,
                    )
                ops.append(oph)
            # normalize: recip of qk column, scale, transpose back
            rc = sc_pool.tile([P, HPC], f32, tag="rc")
            for h in range(HPC):
                nc.vector.reciprocal(rc[:, h : h + 1], ops[h][:, D : D + 1])
            osb = sc_pool.tile([P, P], f16, tag="osb")
            for h in range(HPC):
                nc.vector.tensor_scalar(
                    osb[:, h * D : (h + 1) * D], ops[h][:, 0:D],
                    rc[:, h : h + 1], None, Alu.mult,
                )
            otp = tp_psum.tile([P, P], f16, tag="tp", name=f"otp_{c}")
            nc.tensor.transpose(otp[:], osb[:], id_sb[:])
            ott = ot_pool.tile([P, P], f16, tag="ott")
            nc.vector.tensor_copy(ott[:], otp[:])
            # final projection + store (copy halves split vector/scalar)
            ob = out_pool.tile([P, D_MODEL], f16, tag="ob")
            fps0 = big_psum.tile([P, ST], f32, tag="big", name=f"f0_{c}")
            nc.tensor.matmul(fps0[:], ott[:], wc_sb[:, 0:ST], start=True, stop=True)
            fps1 = big_psum.tile([P, ST], f32, tag="big", name=f"f1_{c}")
            nc.tensor.matmul(fps1[:], ott[:], wc_sb[:, ST:D_MODEL], start=True, stop=True)
            nc.scalar.activation(ob[:, 0:ST], fps0[:], AF.Copy)
            nc.vector.tensor_copy(ob[:, ST:D_MODEL], fps1[:])
            nc.sync.dma_start(out[c * C : (c + 1) * C, :], ob[:])

        def emit_chunks(st):
            for c in range(4 * st, 4 * st + 4):
                emit_indep(c)
            for c in range(4 * st, 4 * st + 4):
                emit_at(c)
                emit_o(c)

        # ---- interleaved emission: proj(st) | chunks(st-1) | feat(st) ----
        for st in range(NST):
            pq = emit_proj("q", st)
            pk = emit_proj("k", st)
            tq, tq2 = emit_tmp("q", pq, st)
            tk, tk2 = emit_tmp("k", pk, st)
            pv = emit_proj("v", st)
            emit_post_v(pv, st)
            if st > 0:
                emit_chunks(st - 1)
            emit_feat("q", tq, tq2, st)
            emit_feat("k", tk, tk2, st)
        emit_chunks(NST - 1)

    nc.finalize()
    return nc


def _prep_inputs(v, k, q, wq_w, wq_b, wk_w, wk_b, wv_w, wv_b, wc_w, wc_b, wg):
    f16 = np.float16
    qT = np.ascontiguousarray(q[0].T).astype(f16)
    kT = np.ascontiguousarray(k[0].T).astype(f16)
    vT = np.ascontiguousarray(v[0].T).astype(f16)
    ident = np.eye(P, dtype=f16)
    mask = np.triu(np.ones((P, P), np.float32)).astype(f16)  # mask[j,i]=1 iff j<=i
    ng = np.full((P, R), -0.5, f16)
    aux = np.concatenate([ident, mask, ng], axis=1)  # (128, 320)

    # fold wg into q/k projections (wg orthogonal: ||x@wg|| == ||x||)
    wg32 = wg.astype(np.float32)
    def fold(w, b):
        wf = np.zeros((D_MODEL, D_MODEL), np.float32)
        bf = np.zeros(D_MODEL, np.float32)
        for h in range(N_HEADS):
            sl = slice(h * D, (h + 1) * D)
            wf[:, sl] = (w[:, sl].astype(np.float32) * NORM_D) @ wg32
            bf[sl] = (b[sl].astype(np.float32) * NORM_D) @ wg32
        return wf, bf
    wqg, bqg = fold(wq_w, wq_b)
    wkg, bkg = fold(wk_w, wk_b)

    in_maps = []
    for c in range(N_CORES):
        cs = slice(c * CW, (c + 1) * CW)
        bqkv = np.stack([
            bqg[cs].astype(np.float32),
            bkg[cs].astype(np.float32),
            wv_b[cs].astype(np.float32),
        ], axis=1)
        in_maps.append({
            "qT": qT, "kT": kT, "vT": vT,
            "wq": wqg[:, cs].astype(f16),
            "wk": wkg[:, cs].astype(f16),
            "wv": wv_w[:, cs].astype(f16),
            "bqkv": bqkv,
            "aux": aux,
            "wc": wc_w[cs, :].astype(f16),
        })
    return in_maps


def kernel(**inputs):
    from concourse.bass_utils import run_bass_kernel_spmd

    if "nc" not in _CACHE:
        _CACHE["nc"] = _build_bass()
    nc = _CACHE["nc"]
    in_maps = _prep_inputs(**inputs)
    res = run_bass_kernel_spmd(nc, in_maps, core_ids=list(range(N_CORES)))
    _CACHE["last_results"] = res
    acc = np.zeros((S, D_MODEL), np.float32)
    for c in range(N_CORES):
        acc += res.results[c]["out"].astype(np.float32)
    acc += inputs["wc_b"].astype(np.float32)[None, :]
    return acc[None, :, :]


if __name__ == "__main__":
    import reference

    inp = {k: np.asarray(v) for k, v in reference.setup_inputs().items()}
    got = kernel(**inp)
    print("kernel out", got.shape, got.dtype)
